# revision 32
# baseline (speedup 1.0000x reference)
"""Dense dot-product attention with key-length masking on 8 Trainium2 cores.

Problem: q,k,v [16, 2048, 128] fp32, valid_lens [16,1] int32.
  out = softmax(mask(q@k.T/sqrt(d))) @ v   (masked keys -> -1e6 before softmax)

v2 of the PSUM-drain-balanced design (see kernel_baseline.py for v1):

- S^T tiles (keys on partitions) from PE; fp16 operands, host pre-transposed.
- ~50% of key tiles drain via ScalarE exp(); the rest via DVE Schraudolph
  exp (round(S*sc+bi) into int16 whose bitcast IS fp16 exp(S/sqrt(d))).
  Both paths mask through per-partition scale/bias vectors.
- O^T accumulates over key tiles with V stationary, E moving (fp16).
- HAM warm-up: a ~4.7us stream of tiny N=128 matmuls runs while the input
  DMAs are in flight, so the PE clock-gate (K=4/8 cold at 1.2GHz) flips to
  8/8 BEFORE the first real matmul (v1 lost ~3us to a cold start).
- Denominators: half-offset in-place pair-sums et[j] += et[m+j] emitted
  INCREMENTALLY in batched multi-tile chunks (GpSimd for big slots, DVE for
  the rest), each chunk's esum DMA issued as soon as its adds complete, so
  no end-of-slot burst ever tails the kernel.
- oT is copied PSUM->SBUF in two [128,512] halves, each issued right after
  the last O matmul writing that half, so the next slot's O accumulation
  (which reuses the same PSUM banks) only waits ~0.6us, hidden by the S
  lookahead.  oT DMAs ride the sync queue (v1 burned ~2.6us of ScalarE
  queue time issuing them).
- The final 1-tile slot's numerator is computed on the HOST from the raw
  E tile it ships anyway (for the denominator): its O matmuls, oT copy and
  oT DMA vanish, and its single S tile is scheduled mid-stream so nothing
  but a small esum DMA trails the last matmul.
- Work distribution: a solver packs (batch, q-half, key-range) segments
  into NCORES x m slots (sizes shared across cores, SPMD), minimizing
  total key tiles; L==0 batches are pure mean(v) on the host.
"""

import math
import sys
import types

import numpy as np

import concourse.bass as bass
import concourse.mybir as mybir
import concourse.tile as tile
from concourse import bacc
from concourse.bass_utils import run_bass_kernel_spmd

B, Q, K, D = 16, 2048, 2048, 128
NCORES = 8
QCH = 1024         # queries per work unit
UNITS = B * (Q // QCH)
NSLOT = UNITS // NCORES
MM_N = 512         # moving-operand free dim per matmul
KT = K // 128      # max key tiles
SCALE = 1.0 / math.sqrt(D)
LOG2E = 1.4426950408889634
SCHC = 0.0574      # Schraudolph shift: zero-mean vs true exp under softmax
WARMUP_MMS = 28    # tiny dummy matmuls to lift the PE HAM clock-gate
R_ACT = 0.50       # fraction of key tiles drained by ScalarE exp()

F32 = mybir.dt.float32
F16 = mybir.dt.float16
I16 = mybir.dt.int16
BF16 = mybir.dt.bfloat16


def _install_hook_stub():
    """bass_utils' axon trace path imports antenv.axon_hooks, which is not
    shipped in this container.  Provide a no-op stub so an ambient
    BASS_TRACE=1 doesn't crash; test harnesses may overwrite the hook."""
    if "antenv.axon_hooks" in sys.modules:
        return
    mod = types.ModuleType("antenv.axon_hooks")
    _hook = [None]
    mod.set_axon_ntff_profile_hook = lambda h: _hook.__setitem__(0, h)
    mod.get_axon_ntff_profile_hook = lambda: _hook[0]
    sys.modules["antenv.axon_hooks"] = mod


_install_hook_stub()

_build_cache = {}
last_result = None  # BassKernelResults of the most recent run (for harnesses)


def _plan_slots(needs):
    """Partition the work (per-unit key-tile counts, units splittable at any
    key-tile boundary because the host sums partial numerators/denominators)
    into NCORES x m segments with per-slot sizes shared across cores (SPMD).
    Returns (sizes, assign) where assign[core][slot] = (b, h, off, len) or
    None; minimizes sum(sizes) (everything scales with it)."""
    total = sum(n for n, _, _ in needs)
    lo = -(-total // NCORES)
    nmax = max(n for n, _, _ in needs)

    def try_pack(sizes):
        rem = sorted([[n, b, h, 0] for n, b, h in needs], reverse=True)
        segs = [[None] * len(sizes) for _ in range(NCORES)]
        free = []
        for j, t in enumerate(sizes):
            for c in range(NCORES):
                free.append([t, j, c])
        for _ in range(10000):
            rem = [r for r in rem if r[0] > 0]
            if not rem:
                return segs
            rem.sort(key=lambda r: -r[0])
            u = rem[0]
            n = u[0]
            if not free:
                return None
            exact = [f for f in free if f[0] <= n]
            if exact:
                f = max(exact, key=lambda f: f[0])
            else:
                f = min(free, key=lambda f: f[0])
            free.remove(f)
            ln = min(n, f[0])
            segs[f[2]][f[1]] = (u[1], u[2], u[3], ln)
            u[3] += ln
            u[0] -= ln
        return None

    sizes_list = []

    def enum(parts, maxp):
        if len(parts) > 6:
            return
        if lo <= sum(parts) <= lo + 4 and parts:
            sizes_list.append(tuple(parts))
        if sum(parts) >= lo + 4:
            return
        for p in range(min(maxp, lo + 4 - sum(parts)), 0, -1):
            parts.append(p)
            enum(parts, p)
            parts.pop()

    enum([], min(16, nmax + 2))
    sizes_list.sort(key=lambda s: (sum(s), len(s)))
    for sizes in sizes_list:
        if sizes[0] < 2:
            continue
        segs = try_pack(list(sizes))
        if segs is not None:
            return list(sizes), segs
    raise RuntimeError("no feasible slot plan")


def _tile_order(trips, hosted):
    """Global tile processing order: non-hosted slots largest-first, hosted
    slots' tiles interleaved early-mid so their raw ships never tail."""
    tiles = []
    for s in range(len(trips)):
        if not hosted[s]:
            tiles.extend((s, i) for i in range(trips[s]))
    pos = 6
    for s in range(len(trips)):
        if hosted[s]:
            for i in range(trips[s]):
                tiles.insert(min(pos, len(tiles)), (s, i))
                pos += 2
    return tiles


def _build(trips, act_sets, pair_eng, nfull, hosted):
    """One SPMD program: slot j processes trips[j] key tiles of one unit.
    Tiles in act_sets[j] drain via ScalarE exp(), the rest via DVE
    Schraudolph; both paths mask via per-partition scale/bias vectors.
    E tiles live in one contiguous [128, t, QCH] tile per slot; the
    denominator ships as half-offset in-place pair sums (et[j] += et[m+j],
    m=ceil(t/2)) computed incrementally in chunks on pair_eng[s] ('g' =
    GpSimd, 'v' = DVE, '' = ship raw).  hosted[s] slots skip the O matmul
    and oT entirely (host computes the numerator from the raw E ship)."""
    nc = bacc.Bacc(num_devices=NCORES)

    nslot = len(trips)
    t_all = list(trips)
    m_all = [(t + 1) // 2 for t in t_all]
    xlens = [QCH + (128 if hosted[s] else 256) * t_all[s] for s in range(nslot)]
    # 'g' slots ship ceil(t/2) pair-summed tiles; all others ship raw
    nship = [m_all[s] if pair_eng[s] == "g" else t_all[s] for s in range(nslot)]
    inbs = [
        nc.declare_dram_parameter(f"inb{s}", [128, xlens[s]], F16, isOutput=False)
        for s in range(nslot)
    ]
    scbi = nc.declare_dram_parameter("scbi", [128, nslot * 4 * KT], F32, isOutput=False)
    n_ot = sum(0 if hosted[s] else 1 for s in range(nslot))
    oT = nc.declare_dram_parameter("oT", [n_ot, 128, QCH], F16, isOutput=True)
    esums = [
        nc.declare_dram_parameter(f"esum{s}", [128, nship[s], QCH], F16, isOutput=True)
        for s in range(nslot)
    ]

    with tile.TileContext(nc) as tc:
        with (
            tc.tile_pool(name="small", bufs=2) as small,
            tc.tile_pool(name="inputs", bufs=1) as inpool,
            tc.tile_pool(name="epool", bufs=1) as epool,
            tc.tile_pool(name="sps", bufs=3, space="PSUM") as pspool,
            tc.tile_pool(name="oacc", bufs=1, space="PSUM") as psacc,
        ):
            # --- HAM warm-up source: tiny stationary so LDW is ~free.
            # Both memsets go BEFORE the gpsimd-queue DMA issues so the
            # warm-up matmuls can start ~0.5us into the kernel.
            wsrc = small.tile([128, 16], BF16)
            nc.gpsimd.memset(wsrc[:], 1.0)
            wmov = small.tile([128, 128], BF16)
            nc.gpsimd.memset(wmov[:], 1.0)

            # Input DMA plan.  All three DMA-capable queues (sync, scalar,
            # gpsimd) share ~358 GB/s of HBM bandwidth and each serializes
            # its own transfers, so pieces are laid out GLOBALLY in
            # earliest-deadline-first order: slot0's whole pack is split in
            # parallel chunks across all queues (its later k/v tiles used
            # to arrive last and stall the PE at tile ~8), then the hosted
            # slots (used early-mid stream), then slots 1+ in stream order.
            inb_tiles = []
            for s in range(nslot):
                inb = inpool.tile([128, xlens[s]], F16, tag=f"inb{s}")
                inb_tiles.append(inb)
            sc_sb = small.tile([128, nslot * 4 * KT], F32)

            tiles_order = _tile_order(trips, hosted)
            first_use = {}
            for pos, (s, i) in enumerate(tiles_order):
                first_use.setdefault(s, pos)
            inb0 = inb_tiles[0]
            vb0 = QCH + 128 * t_all[0]
            kv0 = xlens[0] - QCH  # k+v cols of slot 0
            c1 = QCH + (kv0 // 3) // 128 * 128
            c2 = QCH + (2 * kv0 // 3) // 128 * 128
            nc.sync.dma_start(out=inb0[:, 0:MM_N], in_=inbs[0][:, 0:MM_N])
            nc.gpsimd.dma_start(out=inb0[:, QCH:c1], in_=inbs[0][:, QCH:c1])
            nc.scalar.dma_start(out=inb0[:, MM_N:QCH], in_=inbs[0][:, MM_N:QCH])
            nc.sync.dma_start(out=inb0[:, c1:c2], in_=inbs[0][:, c1:c2])
            nc.scalar.dma_start(out=sc_sb[:], in_=scbi[:])
            nc.scalar.dma_start(out=inb0[:, c2:], in_=inbs[0][:, c2:])
            rot = [nc.gpsimd, nc.sync, nc.scalar]
            ri = 0
            for s in sorted(range(1, nslot), key=lambda s: first_use[s]):
                inb = inb_tiles[s]
                if hosted[s]:
                    rot[ri % 3].dma_start(out=inb[:], in_=inbs[s][:])
                    ri += 1
                else:
                    rot[ri % 3].dma_start(
                        out=inb[:, QCH:], in_=inbs[s][:, QCH:]
                    )
                    rot[(ri + 1) % 3].dma_start(
                        out=inb[:, 0:QCH], in_=inbs[s][:, 0:QCH]
                    )
                    ri += 2

            # --- HAM warm-up: ~4.7us of tiny matmuls while DMAs stream.
            # Stationary is only 16 columns so LDWEIGHTS is ~13ns; each MM
            # streams N=128 (~107ns cold), keeping the PE continuously busy
            # until the HAM clock-gate flips to 8/8.
            wps = pspool.tile([128, QCH], F32, tag="s")
            for w in range(WARMUP_MMS):
                nc.tensor.matmul(
                    wps[:16, 0:128],
                    wsrc[:],
                    wmov[:],
                    start=True,
                    stop=True,
                    skip_group_check=True,
                )

            # per-slot contexts (E tiles and O accumulators allocated up
            # front; deps attach at instruction emission, not allocation)
            ets = []
            o_pss = []
            for s in range(nslot):
                et_s = epool.tile([128, t_all[s], QCH], F16, tag=f"e{s}")
                ets.append(et_s)
                if hosted[s]:
                    o_pss.append(None)
                else:
                    o_ps = psacc.tile([128, QCH], F32, tag="o")
                    o_pss.append(o_ps)

            # software-pipeline the PE queue globally (S matmuls LOOK tiles
            # ahead of the O matmuls, across slot boundaries)
            LOOK = 3  # = sps pool depth
            sps_ref = {}

            def emit_s(s, i):
                s_ps = pspool.tile([128, QCH], F32, tag="s")
                sps_ref[(s, i)] = s_ps
                inb = inb_tiles[s]
                for h in range(QCH // MM_N):
                    nc.tensor.matmul(
                        s_ps[:, bass.ts(h, MM_N)],
                        inb[:, QCH + i * 128 : QCH + (i + 1) * 128],
                        inb[:, bass.ts(h, MM_N)],
                        start=True,
                        stop=True,
                    )

            # tile order: slots largest-first, hosted slots' tiles inserted
            # early-mid (their drain + raw esum ship must not tail the
            # kernel); the stream ends with a raw-shipping slot so nothing
            # but one small DMA trails the last matmul.
            tiles = _tile_order(trips, hosted)

            # denominator plan for paired slots: adjacent-pair in-place adds
            # et[j] = et[2j] + et[2j+1] (2D APs - the batched 3D form
            # measured 2x slower), each emitted right after tile 2j+1 so
            # the add work spreads across the slot.  'g' = GpSimd (slowest,
            # biggest slot), 'v' = DVE singles slotted between its drains.
            # ship[(s,i)] = (j0, j1) esum DMA chunks after tile (s,i).
            pair_at = {}
            ship_at = {}
            for s in range(nslot):
                t, m = t_all[s], m_all[s]
                if pair_eng[s] not in ("g", "v") or t < 2:
                    continue
                for j in range(t // 2):
                    pair_at[(s, 2 * j + 1)] = j
                mid = (t // 2) // 2 if pair_eng[s] == "g" else 0
                if mid > 0:
                    ship_at.setdefault((s, 2 * mid - 1), []).append((0, mid))
                ship_at.setdefault((s, t - 1), []).append((mid, t // 2))
            for k in range(min(LOOK, len(tiles))):
                emit_s(*tiles[k])
            for k, (s, i) in enumerate(tiles):
                t = t_all[s]
                m = m_all[s]
                inb = inb_tiles[s]
                vbase = QCH + 128 * t
                scoff = s * 4 * KT
                et = ets[s]
                s_ps = sps_ref.pop((s, i))
                # tiles fully valid on every core of the slot use immediate
                # scale/bias (no per-partition operand fetch); the rest mask
                # through the per-partition vectors
                if i in act_sets[s]:
                    if i < nfull[s]:
                        nc.scalar.activation(
                            et[:, i, :],
                            s_ps[:],
                            mybir.ActivationFunctionType.Exp,
                            scale=float(SCALE),
                        )
                    else:
                        nc.scalar.activation(
                            et[:, i, :],
                            s_ps[:],
                            mybir.ActivationFunctionType.Exp,
                            bias=sc_sb[:, scoff + KT + i : scoff + KT + i + 1],
                            scale=sc_sb[:, scoff + i : scoff + i + 1],
                        )
                elif i < nfull[s]:
                    nc.vector.tensor_scalar(
                        et[:, i, :].bitcast(I16),
                        s_ps[:],
                        float(SCALE * LOG2E * 1024.0),
                        float((15.0 - SCHC) * 1024.0),
                        mybir.AluOpType.mult,
                        mybir.AluOpType.add,
                    )
                else:
                    nc.vector.tensor_scalar(
                        et[:, i, :].bitcast(I16),
                        s_ps[:],
                        sc_sb[:, scoff + 2 * KT + i : scoff + 2 * KT + i + 1],
                        sc_sb[:, scoff + 3 * KT + i : scoff + 3 * KT + i + 1],
                        mybir.AluOpType.mult,
                        mybir.AluOpType.add,
                    )
                if not hosted[s]:
                    for h in range(QCH // MM_N):
                        nc.tensor.matmul(
                            o_pss[s][:, bass.ts(h, MM_N)],
                            inb[:, vbase + i * 128 : vbase + (i + 1) * 128],
                            et[:, i, bass.ts(h, MM_N)],
                            start=(i == 0),
                            stop=(i == t - 1),
                        )
                if k + LOOK < len(tiles):
                    emit_s(*tiles[k + LOOK])

                # incremental denominator work
                j = pair_at.get((s, i))
                if j is not None:
                    eng = nc.gpsimd if pair_eng[s] == "g" else nc.vector
                    eng.tensor_add(
                        et[:, j, :], et[:, 2 * j, :], et[:, 2 * j + 1, :]
                    )
                for (j0, j1) in ship_at.get((s, i), ()):
                    shq = nc.gpsimd if pair_eng[s] == "g" else nc.sync
                    shq.dma_start(
                        out=esums[s][:, j0:j1, :], in_=et[:, j0:j1, :]
                    )
                    if j1 == t // 2 and t % 2:
                        # odd raw tail tile ships into the last esum slot
                        shq.dma_start(
                            out=esums[s][:, t // 2 : t // 2 + 1, :],
                            in_=et[:, t - 1 : t, :],
                        )
                if pair_eng[s] != "g":
                    # raw-shipping slot: the final slot ships per-tile (so
                    # only one small DMA trails the last matmuls); other raw
                    # slots ship once to keep the DMA/semaphore count down
                    if s == tiles[-1][0]:
                        nc.sync.dma_start(
                            out=esums[s][:, i : i + 1, :],
                            in_=et[:, i : i + 1, :],
                        )
                    elif i == t - 1:
                        nc.sync.dma_start(out=esums[s][:], in_=et[:, :, :])

                if not hosted[s] and i == t - 1:
                    # oT copy in halves right after the last O matmuls; the
                    # next slot's O accumulation reuses these banks, so each
                    # half frees as soon as its copy lands.
                    o_sb = small.tile([128, QCH], F16, tag="osb")
                    oti = sum(0 if hosted[x] else 1 for x in range(s))
                    last = s == tiles[-1][0]
                    for h in range(QCH // MM_N):
                        if last and h == 1:
                            # final slot: copy halves on BOTH drain engines
                            # in parallel to halve the post-stream tail
                            nc.vector.tensor_scalar(
                                o_sb[:, bass.ts(h, MM_N)],
                                o_pss[s][:, bass.ts(h, MM_N)],
                                1.0,
                                0.0,
                                mybir.AluOpType.mult,
                                mybir.AluOpType.add,
                            )
                        else:
                            nc.scalar.copy(
                                o_sb[:, bass.ts(h, MM_N)],
                                o_pss[s][:, bass.ts(h, MM_N)],
                            )
                        if last:
                            nc.sync.dma_start(
                                out=oT[oti, :, bass.ts(h, MM_N)],
                                in_=o_sb[:, bass.ts(h, MM_N)],
                            )
                    if not last:
                        nc.sync.dma_start(out=oT[oti], in_=o_sb[:])

    nc.compile()
    return nc


def kernel(q, k, v, valid_lens):
    q = np.ascontiguousarray(q, dtype=np.float32)
    k = np.ascontiguousarray(k, dtype=np.float32)
    v = np.ascontiguousarray(v, dtype=np.float32)
    L = np.asarray(valid_lens).reshape(-1).astype(np.int64)

    # per-batch key-tile need; L==0 batches are handled entirely on the host
    # (uniform softmax over all keys == plain mean of v)
    need = np.minimum(KT, (L + 127) // 128).astype(np.int64)

    needs = [
        (int(need[b]), b, h)
        for b in range(B)
        for h in range(Q // QCH)
        if need[b] > 0
    ]
    sizes, segs = _plan_slots(needs)
    # largest slot first; smallest last
    order = sorted(range(len(sizes)), key=lambda j: -sizes[j])
    trips = tuple(sizes[j] for j in order)
    assign = [[segs[c][j] for j in order] for c in range(NCORES)]
    nslot = len(trips)
    # host the numerator of the tiny remainder slots (<=2 tiles): they ship
    # raw E for the denominator anyway, and their O matmuls / oT copies /
    # oT DMAs would otherwise tail the kernel
    hosted = tuple(trips[s] <= 2 for s in range(nslot))
    # denominator: GpSimd pair-sums the largest slot (it was 95% busy with
    # two slots), DVE the second-largest (it has drain headroom); every
    # other slot ships raw E tiles
    pair_eng = ["" for _ in range(nslot)]
    by_size = sorted(range(nslot), key=lambda s: -trips[s])
    for rank, s in enumerate(by_size):
        if not hosted[s] and trips[s] // 2 >= 2 and rank < 2:
            pair_eng[s] = "g" if rank == 0 else "v"
    pair_eng = tuple(pair_eng)
    # drain split: ScalarE takes ~R_ACT of the key tiles, interleaved
    # GLOBALLY over the actual tile processing order (per-slot rounding
    # starved ScalarE to 11/32 and overloaded DVE)
    acc = 0.0
    act_lists = [set() for _ in range(nslot)]
    for (s, i) in _tile_order(trips, hosted):
        acc += R_ACT
        if acc >= 1.0:
            acc -= 1.0
            act_lists[s].add(i)
    act_sets = tuple(frozenset(x) for x in act_lists)

    # per-slot count of leading tiles fully valid on EVERY core's segment
    nfull = []
    for s in range(nslot):
        lim = trips[s]
        for c in range(NCORES):
            seg = assign[c][s]
            if seg is None:
                lim = 0
                break
            b, h, off, ln = seg
            lim = min(lim, ln, max(0, int(L[b]) // 128 - off))
        nfull.append(lim)
    nfull = tuple(nfull)

    key = (trips, act_sets, pair_eng, nfull, hosted)
    if key not in _build_cache:
        _build_cache[key] = _build(trips, act_sets, pair_eng, nfull, hosted)
    nc = _build_cache[key]

    qh = q.astype(np.float16)
    kh = k.astype(np.float16)
    vh = v.astype(np.float16)

    # Schraudolph scale/bias per (key-tile, partition): for valid keys
    #   t16 = S*(SCALE*log2e*1024) + (15-C)*1024 ; int16(t16) bitcast fp16
    # masked keys get scale=bias=0 -> +0.0 exactly.
    kidx = np.arange(K)
    scE_all = np.zeros((B, 128, KT), np.float32)
    biE_all = np.full((B, 128, KT), -30.0, np.float32)
    sc2_all = np.zeros((B, 128, KT), np.float32)
    bi2_all = np.zeros((B, 128, KT), np.float32)
    svals = np.float32(SCALE * LOG2E * 1024.0)
    bvals = np.float32((15.0 - SCHC) * 1024.0)
    for b in range(B):
        lb = int(L[b])
        if lb == 0:
            continue
        m = (kidx < lb).astype(np.float32)
        scE_all[b] = (m * np.float32(SCALE)).reshape(KT, 128).T
        biE_all[b] = ((1.0 - m) * np.float32(-30.0)).reshape(KT, 128).T
        sc2_all[b] = (m * svals).reshape(KT, 128).T
        bi2_all[b] = (m * bvals).reshape(KT, 128).T

    in_maps = []
    for c in range(NCORES):
        im = {}
        scbi = np.zeros((128, nslot * 4 * KT), np.float32)
        for s in range(nslot):
            t = trips[s]
            seg = assign[c][s]
            xlen = QCH + (128 if hosted[s] else 256) * t
            pack = np.zeros((128, xlen), np.float16)
            if seg is not None:
                b, h, off, ln = seg
                pack[:, :QCH] = qh[b, h * QCH : (h + 1) * QCH].T
                k0, k1 = off * 128, (off + ln) * 128
                pack[:, QCH : QCH + 128 * ln] = kh[b, k0:k1].T
                if not hosted[s]:
                    # v permuted: partition = key-within-tile, cols = (tile, d)
                    pack[:, QCH + 128 * t : QCH + 128 * (t + ln)] = (
                        vh[b, k0:k1]
                        .reshape(ln, 128, D)
                        .transpose(1, 0, 2)
                        .reshape(128, -1)
                    )
                o = s * 4 * KT
                scbi[:, o : o + ln] = scE_all[b][:, off : off + ln]
                scbi[:, o + KT : o + KT + ln] = biE_all[b][:, off : off + ln]
                scbi[:, o + 2 * KT : o + 2 * KT + ln] = sc2_all[b][:, off : off + ln]
                scbi[:, o + 3 * KT : o + 3 * KT + ln] = bi2_all[b][:, off : off + ln]
            # padding tiles (i >= ln) keep scE=0/biE=-30 and sc2=bi2=0 -> E=0
            ln0 = 0 if seg is None else seg[3]
            scbi[:, s * 4 * KT + KT + ln0 : s * 4 * KT + 2 * KT] = -30.0
            im[f"inb{s}"] = np.ascontiguousarray(pack)
        im["scbi"] = scbi
        in_maps.append(im)

    res = run_bass_kernel_spmd(nc, in_maps, list(range(NCORES)))
    global last_result
    last_result = res

    num = np.zeros((B, Q // QCH, 128, QCH), np.float32)
    den = np.zeros((B, Q // QCH, QCH), np.float32)
    for c in range(NCORES):
        r = res.results[c]
        oti = 0
        for s in range(nslot):
            seg = assign[c][s]
            if seg is None:
                if not hosted[s]:
                    oti += 1
                continue
            b, h, off, ln = seg
            es = r[f"esum{s}"].astype(np.float32)
            den[b, h] += es.sum(axis=(0, 1))
            if hosted[s]:
                # numerator for hosted slots: V_seg^T @ E on the host
                k0 = off * 128
                for i in range(ln):
                    num[b, h] += (
                        v[b, k0 + 128 * i : k0 + 128 * (i + 1)].T @ es[:, i, :]
                    )
            else:
                num[b, h] += r["oT"][oti].astype(np.float32)
                oti += 1
    out = np.empty((B, Q, D), np.float32)
    for b in range(B):
        if L[b] == 0:
            out[b] = v[b].mean(axis=0)[None, :]
            continue
        for h in range(Q // QCH):
            out[b, h * QCH : (h + 1) * QCH] = (num[b, h] / den[b, h][None, :]).T
    return out


# revision 33
# speedup vs baseline: 1.2388x; 1.2388x over previous
"""Dense dot-product attention with key-length masking on 8 Trainium2 cores.

Problem: q,k,v [16, 2048, 128] fp32, valid_lens [16,1] int32.
  out = softmax(mask(q@k.T/sqrt(d))) @ v   (masked keys -> -1e6 before softmax)

v2 of the PSUM-drain-balanced design (see kernel_baseline.py for v1):

- S^T tiles (keys on partitions) from PE; fp16 operands, host pre-transposed.
- ~50% of key tiles drain via ScalarE exp(); the rest via DVE Schraudolph
  exp (round(S*sc+bi) into int16 whose bitcast IS fp16 exp(S/sqrt(d))).
  Both paths mask through per-partition scale/bias vectors.
- O^T accumulates over key tiles with V stationary, E moving (fp16).
- HAM warm-up: a ~4.7us stream of tiny N=128 matmuls runs while the input
  DMAs are in flight, so the PE clock-gate (K=4/8 cold at 1.2GHz) flips to
  8/8 BEFORE the first real matmul (v1 lost ~3us to a cold start).
- Denominators: half-offset in-place pair-sums et[j] += et[m+j] emitted
  INCREMENTALLY in batched multi-tile chunks (GpSimd for big slots, DVE for
  the rest), each chunk's esum DMA issued as soon as its adds complete, so
  no end-of-slot burst ever tails the kernel.
- oT is copied PSUM->SBUF in two [128,512] halves, each issued right after
  the last O matmul writing that half, so the next slot's O accumulation
  (which reuses the same PSUM banks) only waits ~0.6us, hidden by the S
  lookahead.  oT DMAs ride the sync queue (v1 burned ~2.6us of ScalarE
  queue time issuing them).
- The final 1-tile slot's numerator is computed on the HOST from the raw
  E tile it ships anyway (for the denominator): its O matmuls, oT copy and
  oT DMA vanish, and its single S tile is scheduled mid-stream so nothing
  but a small esum DMA trails the last matmul.
- Work distribution: a solver packs (batch, q-half, key-range) segments
  into NCORES x m slots (sizes shared across cores, SPMD), minimizing
  total key tiles; L==0 batches are pure mean(v) on the host.
"""

import math
import sys
import types

import numpy as np

import concourse.bass as bass
import concourse.mybir as mybir
import concourse.tile as tile
from concourse import bacc
from concourse.bass_utils import run_bass_kernel_spmd

B, Q, K, D = 16, 2048, 2048, 128
NCORES = 8
QCH = 1024         # queries per work unit
UNITS = B * (Q // QCH)
NSLOT = UNITS // NCORES
MM_N = 512         # moving-operand free dim per matmul
KT = K // 128      # max key tiles
SCALE = 1.0 / math.sqrt(D)
LOG2E = 1.4426950408889634
SCHC = 0.0574      # Schraudolph shift: zero-mean vs true exp under softmax
WARMUP_MMS = 28    # tiny dummy matmuls to lift the PE HAM clock-gate
R_ACT = 0.50       # fraction of key tiles drained by ScalarE exp()

F32 = mybir.dt.float32
F16 = mybir.dt.float16
I16 = mybir.dt.int16
BF16 = mybir.dt.bfloat16


def _install_hook_stub():
    """bass_utils' axon trace path imports antenv.axon_hooks, which is not
    shipped in this container.  Provide a no-op stub so an ambient
    BASS_TRACE=1 doesn't crash; test harnesses may overwrite the hook."""
    if "antenv.axon_hooks" in sys.modules:
        return
    mod = types.ModuleType("antenv.axon_hooks")
    _hook = [None]
    mod.set_axon_ntff_profile_hook = lambda h: _hook.__setitem__(0, h)
    mod.get_axon_ntff_profile_hook = lambda: _hook[0]
    sys.modules["antenv.axon_hooks"] = mod


_install_hook_stub()

_build_cache = {}
last_result = None  # BassKernelResults of the most recent run (for harnesses)


def _plan_slots(needs):
    """Partition the work (per-unit key-tile counts, units splittable at any
    key-tile boundary because the host sums partial numerators/denominators)
    into NCORES x m segments with per-slot sizes shared across cores (SPMD).
    Returns (sizes, assign) where assign[core][slot] = (b, h, off, len) or
    None; minimizes sum(sizes) (everything scales with it)."""
    total = sum(n for n, _, _ in needs)
    lo = -(-total // NCORES)
    nmax = max(n for n, _, _ in needs)

    def try_pack(sizes):
        rem = sorted([[n, b, h, 0] for n, b, h in needs], reverse=True)
        segs = [[None] * len(sizes) for _ in range(NCORES)]
        free = []
        for j, t in enumerate(sizes):
            for c in range(NCORES):
                free.append([t, j, c])
        for _ in range(10000):
            rem = [r for r in rem if r[0] > 0]
            if not rem:
                return segs
            rem.sort(key=lambda r: -r[0])
            u = rem[0]
            n = u[0]
            if not free:
                return None
            exact = [f for f in free if f[0] <= n]
            if exact:
                f = max(exact, key=lambda f: f[0])
            else:
                f = min(free, key=lambda f: f[0])
            free.remove(f)
            ln = min(n, f[0])
            segs[f[2]][f[1]] = (u[1], u[2], u[3], ln)
            u[3] += ln
            u[0] -= ln
        return None

    sizes_list = []

    def enum(parts, maxp):
        if len(parts) > 6:
            return
        if lo <= sum(parts) <= lo + 4 and parts:
            sizes_list.append(tuple(parts))
        if sum(parts) >= lo + 4:
            return
        for p in range(min(maxp, lo + 4 - sum(parts)), 0, -1):
            parts.append(p)
            enum(parts, p)
            parts.pop()

    enum([], min(16, nmax + 2))
    sizes_list.sort(key=lambda s: (sum(s), len(s)))
    for sizes in sizes_list:
        if sizes[0] < 2:
            continue
        segs = try_pack(list(sizes))
        if segs is not None:
            return list(sizes), segs
    raise RuntimeError("no feasible slot plan")


def _tile_order(trips, hosted):
    """Global tile processing order: non-hosted slots largest-first, hosted
    slots' tiles interleaved early-mid so their raw ships never tail."""
    tiles = []
    for s in range(len(trips)):
        if not hosted[s]:
            tiles.extend((s, i) for i in range(trips[s]))
    pos = 6
    for s in range(len(trips)):
        if hosted[s]:
            for i in range(trips[s]):
                tiles.insert(min(pos, len(tiles)), (s, i))
                pos += 2
    return tiles


def _build(trips, act_sets, pair_eng, nfull, hosted):
    """One SPMD program: slot j processes trips[j] key tiles of one unit.
    Tiles in act_sets[j] drain via ScalarE exp(), the rest via DVE
    Schraudolph; both paths mask via per-partition scale/bias vectors.
    E tiles live in one contiguous [128, t, QCH] tile per slot; the
    denominator ships as half-offset in-place pair sums (et[j] += et[m+j],
    m=ceil(t/2)) computed incrementally in chunks on pair_eng[s] ('g' =
    GpSimd, 'v' = DVE, '' = ship raw).  hosted[s] slots skip the O matmul
    and oT entirely (host computes the numerator from the raw E ship)."""
    nc = bacc.Bacc(num_devices=NCORES)

    nslot = len(trips)
    t_all = list(trips)
    m_all = [(t + 1) // 2 for t in t_all]
    xlens = [QCH + (128 if hosted[s] else 256) * t_all[s] for s in range(nslot)]
    # 'g' slots ship ceil(t/2) pair-summed tiles; all others ship raw
    nship = [m_all[s] if pair_eng[s] == "g" else t_all[s] for s in range(nslot)]
    inbs = [
        nc.declare_dram_parameter(f"inb{s}", [128, xlens[s]], F16, isOutput=False)
        for s in range(nslot)
    ]
    scbi = nc.declare_dram_parameter("scbi", [128, nslot * 4 * KT], F32, isOutput=False)
    n_ot = sum(0 if hosted[s] else 1 for s in range(nslot))
    oT = nc.declare_dram_parameter("oT", [n_ot, 128, QCH], F16, isOutput=True)
    esums = [
        nc.declare_dram_parameter(f"esum{s}", [128, nship[s], QCH], F16, isOutput=True)
        for s in range(nslot)
    ]

    with tile.TileContext(nc) as tc:
        with (
            tc.tile_pool(name="small", bufs=2) as small,
            tc.tile_pool(name="inputs", bufs=1) as inpool,
            tc.tile_pool(name="epool", bufs=1) as epool,
            tc.tile_pool(name="sps", bufs=3, space="PSUM") as pspool,
            tc.tile_pool(name="oacc", bufs=1, space="PSUM") as psacc,
        ):
            # --- HAM warm-up source: tiny stationary so LDW is ~free.
            # Both memsets go BEFORE the gpsimd-queue DMA issues so the
            # warm-up matmuls can start ~0.5us into the kernel.
            wsrc = small.tile([128, 16], BF16)
            nc.gpsimd.memset(wsrc[:], 1.0)
            wmov = small.tile([128, 128], BF16)
            nc.gpsimd.memset(wmov[:], 1.0)

            # Input DMA plan.  All three DMA-capable queues (sync, scalar,
            # gpsimd) share ~358 GB/s of HBM bandwidth and each serializes
            # its own transfers, so pieces are laid out GLOBALLY in
            # earliest-deadline-first order: slot0's whole pack is split in
            # parallel chunks across all queues (its later k/v tiles used
            # to arrive last and stall the PE at tile ~8), then the hosted
            # slots (used early-mid stream), then slots 1+ in stream order.
            inb_tiles = []
            for s in range(nslot):
                inb = inpool.tile([128, xlens[s]], F16, tag=f"inb{s}")
                inb_tiles.append(inb)
            sc_sb = small.tile([128, nslot * 4 * KT], F32)

            tiles_order = _tile_order(trips, hosted)
            first_use = {}
            for pos, (s, i) in enumerate(tiles_order):
                first_use.setdefault(s, pos)
            inb0 = inb_tiles[0]
            vb0 = QCH + 128 * t_all[0]
            kv0 = xlens[0] - QCH  # k+v cols of slot 0
            c1 = QCH + (kv0 // 3) // 128 * 128
            c2 = QCH + (2 * kv0 // 3) // 128 * 128
            nc.sync.dma_start(out=inb0[:, 0:MM_N], in_=inbs[0][:, 0:MM_N])
            nc.gpsimd.dma_start(out=inb0[:, QCH:c1], in_=inbs[0][:, QCH:c1])
            nc.scalar.dma_start(out=inb0[:, MM_N:QCH], in_=inbs[0][:, MM_N:QCH])
            nc.sync.dma_start(out=inb0[:, c1:c2], in_=inbs[0][:, c1:c2])
            nc.scalar.dma_start(out=sc_sb[:], in_=scbi[:])
            nc.scalar.dma_start(out=inb0[:, c2:], in_=inbs[0][:, c2:])
            rot = [nc.gpsimd, nc.sync, nc.scalar]
            ri = 0
            for s in sorted(range(1, nslot), key=lambda s: first_use[s]):
                inb = inb_tiles[s]
                if hosted[s]:
                    rot[ri % 3].dma_start(out=inb[:], in_=inbs[s][:])
                    ri += 1
                else:
                    rot[ri % 3].dma_start(
                        out=inb[:, QCH:], in_=inbs[s][:, QCH:]
                    )
                    rot[(ri + 1) % 3].dma_start(
                        out=inb[:, 0:QCH], in_=inbs[s][:, 0:QCH]
                    )
                    ri += 2

            # --- HAM warm-up: ~4.7us of tiny matmuls while DMAs stream.
            # Stationary is only 16 columns so LDWEIGHTS is ~13ns; each MM
            # streams N=128 (~107ns cold), keeping the PE continuously busy
            # until the HAM clock-gate flips to 8/8.
            wps = pspool.tile([128, QCH], F32, tag="s")
            for w in range(WARMUP_MMS):
                nc.tensor.matmul(
                    wps[:16, 0:128],
                    wsrc[:],
                    wmov[:],
                    start=True,
                    stop=True,
                    skip_group_check=True,
                )

            # per-slot contexts (E tiles and O accumulators allocated up
            # front; deps attach at instruction emission, not allocation)
            ets = []
            o_pss = []
            for s in range(nslot):
                et_s = epool.tile([128, t_all[s], QCH], F16, tag=f"e{s}")
                ets.append(et_s)
                if hosted[s]:
                    o_pss.append(None)
                else:
                    o_ps = psacc.tile([128, QCH], F32, tag="o")
                    o_pss.append(o_ps)

            # software-pipeline the PE queue globally (S matmuls LOOK tiles
            # ahead of the O matmuls, across slot boundaries)
            LOOK = 3  # = sps pool depth
            sps_ref = {}

            def emit_s(s, i):
                s_ps = pspool.tile([128, QCH], F32, tag="s")
                sps_ref[(s, i)] = s_ps
                inb = inb_tiles[s]
                for h in range(QCH // MM_N):
                    nc.tensor.matmul(
                        s_ps[:, bass.ts(h, MM_N)],
                        inb[:, QCH + i * 128 : QCH + (i + 1) * 128],
                        inb[:, bass.ts(h, MM_N)],
                        start=True,
                        stop=True,
                    )

            # tile order: slots largest-first, hosted slots' tiles inserted
            # early-mid (their drain + raw esum ship must not tail the
            # kernel); the stream ends with a raw-shipping slot so nothing
            # but one small DMA trails the last matmul.
            tiles = _tile_order(trips, hosted)

            # denominator plan for paired slots: adjacent-pair in-place adds
            # et[j] = et[2j] + et[2j+1] (2D APs - the batched 3D form
            # measured 2x slower), each emitted right after tile 2j+1 so
            # the add work spreads across the slot.  'g' = GpSimd (slowest,
            # biggest slot), 'v' = DVE singles slotted between its drains.
            # ship[(s,i)] = (j0, j1) esum DMA chunks after tile (s,i).
            pair_at = {}
            ship_at = {}
            for s in range(nslot):
                t, m = t_all[s], m_all[s]
                if pair_eng[s] not in ("g", "v") or t < 2:
                    continue
                for j in range(t // 2):
                    pair_at[(s, 2 * j + 1)] = j
                mid = (t // 2) // 2 if pair_eng[s] == "g" else 0
                if mid > 0:
                    ship_at.setdefault((s, 2 * mid - 1), []).append((0, mid))
                ship_at.setdefault((s, t - 1), []).append((mid, t // 2))
            for k in range(min(LOOK, len(tiles))):
                emit_s(*tiles[k])
            for k, (s, i) in enumerate(tiles):
                t = t_all[s]
                m = m_all[s]
                inb = inb_tiles[s]
                vbase = QCH + 128 * t
                scoff = s * 4 * KT
                et = ets[s]
                s_ps = sps_ref.pop((s, i))
                # tiles fully valid on every core of the slot use immediate
                # scale/bias (no per-partition operand fetch); the rest mask
                # through the per-partition vectors
                if i in act_sets[s]:
                    if i < nfull[s]:
                        nc.scalar.activation(
                            et[:, i, :],
                            s_ps[:],
                            mybir.ActivationFunctionType.Exp,
                            scale=float(SCALE),
                        )
                    else:
                        nc.scalar.activation(
                            et[:, i, :],
                            s_ps[:],
                            mybir.ActivationFunctionType.Exp,
                            bias=sc_sb[:, scoff + KT + i : scoff + KT + i + 1],
                            scale=sc_sb[:, scoff + i : scoff + i + 1],
                        )
                elif i < nfull[s]:
                    nc.vector.tensor_scalar(
                        et[:, i, :].bitcast(I16),
                        s_ps[:],
                        float(SCALE * LOG2E * 1024.0),
                        float((15.0 - SCHC) * 1024.0),
                        mybir.AluOpType.mult,
                        mybir.AluOpType.add,
                    )
                else:
                    nc.vector.tensor_scalar(
                        et[:, i, :].bitcast(I16),
                        s_ps[:],
                        sc_sb[:, scoff + 2 * KT + i : scoff + 2 * KT + i + 1],
                        sc_sb[:, scoff + 3 * KT + i : scoff + 3 * KT + i + 1],
                        mybir.AluOpType.mult,
                        mybir.AluOpType.add,
                    )
                if not hosted[s]:
                    for h in range(QCH // MM_N):
                        nc.tensor.matmul(
                            o_pss[s][:, bass.ts(h, MM_N)],
                            inb[:, vbase + i * 128 : vbase + (i + 1) * 128],
                            et[:, i, bass.ts(h, MM_N)],
                            start=(i == 0),
                            stop=(i == t - 1),
                        )
                if k + LOOK < len(tiles):
                    emit_s(*tiles[k + LOOK])

                # incremental denominator work
                j = pair_at.get((s, i))
                if j is not None:
                    eng = nc.gpsimd if pair_eng[s] == "g" else nc.vector
                    eng.tensor_add(
                        et[:, j, :], et[:, 2 * j, :], et[:, 2 * j + 1, :]
                    )
                for (j0, j1) in ship_at.get((s, i), ()):
                    shq = nc.gpsimd if pair_eng[s] == "g" else nc.sync
                    shq.dma_start(
                        out=esums[s][:, j0:j1, :], in_=et[:, j0:j1, :]
                    )
                    if j1 == t // 2 and t % 2:
                        # odd raw tail tile ships into the last esum slot
                        shq.dma_start(
                            out=esums[s][:, t // 2 : t // 2 + 1, :],
                            in_=et[:, t - 1 : t, :],
                        )
                if pair_eng[s] == "":
                    # raw-shipping slot: the final slot ships per-tile (so
                    # only one small DMA trails the last matmuls); other raw
                    # slots ship once to keep the DMA/semaphore count down
                    if s == tiles[-1][0]:
                        nc.sync.dma_start(
                            out=esums[s][:, i : i + 1, :],
                            in_=et[:, i : i + 1, :],
                        )
                    elif i == t - 1:
                        nc.sync.dma_start(out=esums[s][:], in_=et[:, :, :])

                if not hosted[s] and i == t - 1:
                    # oT copy in halves right after the last O matmuls; the
                    # next slot's O accumulation reuses these banks, so each
                    # half frees as soon as its copy lands.
                    o_sb = small.tile([128, QCH], F16, tag="osb")
                    oti = sum(0 if hosted[x] else 1 for x in range(s))
                    last = s == tiles[-1][0]
                    for h in range(QCH // MM_N):
                        if last and h == 1:
                            # final slot: copy halves on BOTH drain engines
                            # in parallel to halve the post-stream tail
                            nc.vector.tensor_scalar(
                                o_sb[:, bass.ts(h, MM_N)],
                                o_pss[s][:, bass.ts(h, MM_N)],
                                1.0,
                                0.0,
                                mybir.AluOpType.mult,
                                mybir.AluOpType.add,
                            )
                        else:
                            nc.scalar.copy(
                                o_sb[:, bass.ts(h, MM_N)],
                                o_pss[s][:, bass.ts(h, MM_N)],
                            )
                        if last:
                            nc.sync.dma_start(
                                out=oT[oti, :, bass.ts(h, MM_N)],
                                in_=o_sb[:, bass.ts(h, MM_N)],
                            )
                    if not last:
                        nc.sync.dma_start(out=oT[oti], in_=o_sb[:])

    nc.compile()
    return nc


def kernel(q, k, v, valid_lens):
    q = np.ascontiguousarray(q, dtype=np.float32)
    k = np.ascontiguousarray(k, dtype=np.float32)
    v = np.ascontiguousarray(v, dtype=np.float32)
    L = np.asarray(valid_lens).reshape(-1).astype(np.int64)

    # per-batch key-tile need; L==0 batches are handled entirely on the host
    # (uniform softmax over all keys == plain mean of v)
    need = np.minimum(KT, (L + 127) // 128).astype(np.int64)

    needs = [
        (int(need[b]), b, h)
        for b in range(B)
        for h in range(Q // QCH)
        if need[b] > 0
    ]
    sizes, segs = _plan_slots(needs)
    # largest slot first; smallest last
    order = sorted(range(len(sizes)), key=lambda j: -sizes[j])
    trips = tuple(sizes[j] for j in order)
    assign = [[segs[c][j] for j in order] for c in range(NCORES)]
    nslot = len(trips)
    # host the numerator of the tiny remainder slots (<=2 tiles): they ship
    # raw E for the denominator anyway, and their O matmuls / oT copies /
    # oT DMAs would otherwise tail the kernel
    hosted = tuple(trips[s] <= 2 for s in range(nslot))
    # denominator: GpSimd pair-sums the largest slot (it was 95% busy with
    # two slots), DVE the second-largest (it has drain headroom); every
    # other slot ships raw E tiles
    pair_eng = ["" for _ in range(nslot)]
    by_size = sorted(range(nslot), key=lambda s: -trips[s])
    for rank, s in enumerate(by_size):
        if not hosted[s] and trips[s] // 2 >= 2 and rank < 2:
            pair_eng[s] = "g" if rank == 0 else "v"
    pair_eng = tuple(pair_eng)
    # drain split: ScalarE takes ~R_ACT of the key tiles, interleaved
    # GLOBALLY over the actual tile processing order (per-slot rounding
    # starved ScalarE to 11/32 and overloaded DVE)
    acc = 0.0
    act_lists = [set() for _ in range(nslot)]
    for (s, i) in _tile_order(trips, hosted):
        acc += R_ACT
        if acc >= 1.0:
            acc -= 1.0
            act_lists[s].add(i)
    act_sets = tuple(frozenset(x) for x in act_lists)

    # per-slot count of leading tiles fully valid on EVERY core's segment
    nfull = []
    for s in range(nslot):
        lim = trips[s]
        for c in range(NCORES):
            seg = assign[c][s]
            if seg is None:
                lim = 0
                break
            b, h, off, ln = seg
            lim = min(lim, ln, max(0, int(L[b]) // 128 - off))
        nfull.append(lim)
    nfull = tuple(nfull)

    key = (trips, act_sets, pair_eng, nfull, hosted)
    if key not in _build_cache:
        _build_cache[key] = _build(trips, act_sets, pair_eng, nfull, hosted)
    nc = _build_cache[key]

    qh = q.astype(np.float16)
    kh = k.astype(np.float16)
    vh = v.astype(np.float16)

    # Schraudolph scale/bias per (key-tile, partition): for valid keys
    #   t16 = S*(SCALE*log2e*1024) + (15-C)*1024 ; int16(t16) bitcast fp16
    # masked keys get scale=bias=0 -> +0.0 exactly.
    kidx = np.arange(K)
    scE_all = np.zeros((B, 128, KT), np.float32)
    biE_all = np.full((B, 128, KT), -30.0, np.float32)
    sc2_all = np.zeros((B, 128, KT), np.float32)
    bi2_all = np.zeros((B, 128, KT), np.float32)
    svals = np.float32(SCALE * LOG2E * 1024.0)
    bvals = np.float32((15.0 - SCHC) * 1024.0)
    for b in range(B):
        lb = int(L[b])
        if lb == 0:
            continue
        m = (kidx < lb).astype(np.float32)
        scE_all[b] = (m * np.float32(SCALE)).reshape(KT, 128).T
        biE_all[b] = ((1.0 - m) * np.float32(-30.0)).reshape(KT, 128).T
        sc2_all[b] = (m * svals).reshape(KT, 128).T
        bi2_all[b] = (m * bvals).reshape(KT, 128).T

    in_maps = []
    for c in range(NCORES):
        im = {}
        scbi = np.zeros((128, nslot * 4 * KT), np.float32)
        for s in range(nslot):
            t = trips[s]
            seg = assign[c][s]
            xlen = QCH + (128 if hosted[s] else 256) * t
            pack = np.zeros((128, xlen), np.float16)
            if seg is not None:
                b, h, off, ln = seg
                pack[:, :QCH] = qh[b, h * QCH : (h + 1) * QCH].T
                k0, k1 = off * 128, (off + ln) * 128
                pack[:, QCH : QCH + 128 * ln] = kh[b, k0:k1].T
                if not hosted[s]:
                    # v permuted: partition = key-within-tile, cols = (tile, d)
                    pack[:, QCH + 128 * t : QCH + 128 * (t + ln)] = (
                        vh[b, k0:k1]
                        .reshape(ln, 128, D)
                        .transpose(1, 0, 2)
                        .reshape(128, -1)
                    )
                o = s * 4 * KT
                scbi[:, o : o + ln] = scE_all[b][:, off : off + ln]
                scbi[:, o + KT : o + KT + ln] = biE_all[b][:, off : off + ln]
                scbi[:, o + 2 * KT : o + 2 * KT + ln] = sc2_all[b][:, off : off + ln]
                scbi[:, o + 3 * KT : o + 3 * KT + ln] = bi2_all[b][:, off : off + ln]
            # padding tiles (i >= ln) keep scE=0/biE=-30 and sc2=bi2=0 -> E=0
            ln0 = 0 if seg is None else seg[3]
            scbi[:, s * 4 * KT + KT + ln0 : s * 4 * KT + 2 * KT] = -30.0
            im[f"inb{s}"] = np.ascontiguousarray(pack)
        im["scbi"] = scbi
        in_maps.append(im)

    res = run_bass_kernel_spmd(nc, in_maps, list(range(NCORES)))
    global last_result
    last_result = res

    num = np.zeros((B, Q // QCH, 128, QCH), np.float32)
    den = np.zeros((B, Q // QCH, QCH), np.float32)
    for c in range(NCORES):
        r = res.results[c]
        oti = 0
        for s in range(nslot):
            seg = assign[c][s]
            if seg is None:
                if not hosted[s]:
                    oti += 1
                continue
            b, h, off, ln = seg
            es = r[f"esum{s}"].astype(np.float32)
            den[b, h] += es.sum(axis=(0, 1))
            if hosted[s]:
                # numerator for hosted slots: V_seg^T @ E on the host
                k0 = off * 128
                for i in range(ln):
                    num[b, h] += (
                        v[b, k0 + 128 * i : k0 + 128 * (i + 1)].T @ es[:, i, :]
                    )
            else:
                num[b, h] += r["oT"][oti].astype(np.float32)
                oti += 1
    out = np.empty((B, Q, D), np.float32)
    for b in range(B):
        if L[b] == 0:
            out[b] = v[b].mean(axis=0)[None, :]
            continue
        for h in range(Q // QCH):
            out[b, h * QCH : (h + 1) * QCH] = (num[b, h] / den[b, h][None, :]).T
    return out
